# revision 13
# baseline (speedup 1.0000x reference)
"""Trainium2 Bass kernel for a 6-layer caption-generator transformer.

Sharding: data-parallel over batch (16 -> 2 per core) for the 6 transformer
layers; vocab-sharded final projection + softmax (50257 -> 6283 cols/core)
with an AllGather of final hidden states and pipelined grouped AllReduces
of the softmax denominators.

Self-contained: hardcodes all shapes; builds + compiles the Bass/Tile
program on first call (cached) and runs it on 8 NeuronCores via
run_bass_kernel_spmd.
"""

import numpy as np
import ml_dtypes
from contextlib import ExitStack

# ---- model dims (hardcoded from the problem spec) ----
B, IMG, TXT = 16, 197, 24
S = IMG + TXT + 1          # 222
D, H, L, V = 768, 12, 6, 50257
HD = D // H                # 64
F = 4 * D                  # 3072
EPS = 1e-5
NCORE = 8
BC = B // NCORE            # 2 batches per core
NT = BC * S                # 444 tokens per core
KD = D // 128              # 6 k-tiles over D
KF = F // 128              # 24 k-tiles over F
ATT_SCALE = 1.0 / float(np.sqrt(np.float32(HD)))

# local (per-batch) token tiles: (offset, len)
LT = [(0, 128), (128, S - 128)]          # [(0,128),(128,94)]
# per-core token tiles within the 444-token block
MT_LOC = [(0, 128), (128, 128), (256, 128), (384, NT - 384)]

# vocab sharding
VC = 6283                   # vocab cols per core (8*6283 = 50264 >= 50257)
V_PAD = VC * NCORE          # 50264
N_VPAD = V_PAD - V          # 7 padded cols (wout=0 -> logits 0 -> exp 1)
VCH = [(i * 512, 512) for i in range(12)] + [(6144, VC - 6144)]   # 12x512+139

N_MT = NCORE * len(MT_LOC)  # 32 token m-tiles over the gathered 3552 tokens
ZGRP = 2                    # m-tiles per Z AllReduce group
N_ZG = N_MT // ZGRP         # 16 groups

_CACHE = {}


def _build(flags):
    import concourse.bass as bass
    import concourse.tile as tile
    import concourse.mybir as mybir
    from concourse import bacc

    f32 = mybir.dt.float32
    f32r = mybir.dt.float32r
    bf16 = mybir.dt.bfloat16
    AF = mybir.ActivationFunctionType
    OP = mybir.AluOpType
    AX = mybir.AxisListType

    bv_nz, b2_nz, bout_nz = flags

    nc = bacc.Bacc("TRN2", target_bir_lowering=False, debug=False,
                   num_devices=NCORE)

    # ---- DRAM I/O ----
    d_x0 = nc.dram_tensor("x0t", [D, NT], f32r, kind="ExternalInput").ap()
    d_wqk = nc.dram_tensor("wqk", [L, D, 2 * D], f32r, kind="ExternalInput").ap()
    d_wv = nc.dram_tensor("wv", [L, D, D], f32r, kind="ExternalInput").ap()
    d_w1 = nc.dram_tensor("w1", [L, D, F], f32r, kind="ExternalInput").ap()
    d_w2 = nc.dram_tensor("w2", [L, F, D], bf16, kind="ExternalInput").ap()
    d_wo = nc.dram_tensor("wout", [D, VC], bf16, kind="ExternalInput").ap()
    d_pp = nc.dram_tensor("pp", [L, 128, 72], f32, kind="ExternalInput").ap()
    d_mask = nc.dram_tensor("maskt", [BC, S, S], bf16, kind="ExternalInput").ap()
    d_ones = nc.dram_tensor("ones", [128, 128], f32r, kind="ExternalInput").ap()
    d_ident = nc.dram_tensor("ident", [128, 128], f32r, kind="ExternalInput").ap()
    d_bvbc = (nc.dram_tensor("bvbc", [L, 128, D], f32, kind="ExternalInput").ap()
              if bv_nz else None)
    d_bout = (nc.dram_tensor("boutbc", [128, VC], f32, kind="ExternalInput").ap()
              if bout_nz else None)
    d_out = nc.dram_tensor("out", [NCORE * NT, VC], f32,
                           kind="ExternalOutput").ap()

    with tile.TileContext(nc) as tc, ExitStack() as top, \
            nc.allow_low_precision(reason="fp32r matmul tiles"):
        # psum pools shared across phases via tags (4+4 banks)
        pps = top.enter_context(tc.tile_pool(name="pps", bufs=4, space="PSUM"))
        ppa = top.enter_context(tc.tile_pool(name="ppa", bufs=4, space="PSUM"))
        pdram = top.enter_context(tc.tile_pool(name="pdram", bufs=1, space="DRAM"))
        pconst = top.enter_context(tc.tile_pool(name="pconst", bufs=1))

        ones_sb = pconst.tile([128, 128], f32r, tag="ones")
        nc.sync.dma_start(ones_sb[:], d_ones[:])
        ident_sb = pconst.tile([128, 128], f32r, tag="ident")
        nc.sync.dma_start(ident_sb[:], d_ident[:])
        eps_sb = pconst.tile([128, 1], f32, tag="eps")
        nc.vector.memset(eps_sb[:], EPS)

        def mmr(out, lhsT, rhs, **kw):
            nc.tensor.matmul(out, lhsT, rhs, **kw)

        # dram staging for collectives
        ag_in = pdram.tile([D, NT], bf16)
        ag_out = pdram.tile([NCORE, D, NT], bf16, addr_space="Shared")
        z_in = [pdram.tile([128, ZGRP], f32, name=f"zin{g}") for g in range(N_ZG)]
        z_out = [pdram.tile([128, ZGRP], f32, name=f"zout{g}",
                            addr_space="Shared") for g in range(N_ZG)]

        with ExitStack() as lay:
            px = lay.enter_context(tc.tile_pool(name="px", bufs=15))
            pqk = lay.enter_context(tc.tile_pool(name="pqk", bufs=13))
            pv = lay.enter_context(tc.tile_pool(name="pv", bufs=5))
            patt = lay.enter_context(tc.tile_pool(name="patt", bufs=6))
            pescr = lay.enter_context(tc.tile_pool(name="pescr", bufs=3))
            pocat = lay.enter_context(tc.tile_pool(name="pocat", bufs=4))
            ph = lay.enter_context(tc.tile_pool(name="ph", bufs=26))
            plns = lay.enter_context(tc.tile_pool(name="plns", bufs=3))
            pmask = lay.enter_context(tc.tile_pool(name="pmask", bufs=4))
            pwmed = lay.enter_context(tc.tile_pool(name="pwmed", bufs=6))
            pwsm = lay.enter_context(tc.tile_pool(name="pwsm", bufs=10))
            pw2 = lay.enter_context(tc.tile_pool(name="pw2", bufs=26))
            ppp = lay.enter_context(tc.tile_pool(name="ppp", bufs=2))
            pst = lay.enter_context(tc.tile_pool(name="pst", bufs=8))
            pbv = (lay.enter_context(tc.tile_pool(name="pbv", bufs=2))
                   if bv_nz else None)

            # initial x (transposed [D, NT])
            xt = []
            for k in range(KD):
                t = px.tile([128, NT], f32r, tag="x")
                nc.sync.dma_start(t[:], d_x0[k * 128:(k + 1) * 128, :])
                xt.append(t)

            # mask tiles (resident; maskT[b][t,s] layout)
            mk = {}
            for b in range(BC):
                for ti, (t0, tl) in enumerate(LT):
                    m = pmask.tile([128, S], bf16, tag="mask")
                    nc.sync.dma_start(m[:tl, :], d_mask[b, t0:t0 + tl, :])
                    mk[(b, ti)] = m

            for l in range(L):
                pp_sb = ppp.tile([128, 72], f32, tag="pp")
                nc.sync.dma_start(pp_sb[:], d_pp[l])

                # ---- Q,K projections (transposed layout [head*hd, tok]) ----
                wq_sb, wk_sb = [], []
                for k in range(KD):
                    t = pwsm.tile([128, D], f32r, tag="wsm")
                    nc.sync.dma_start(t[:], d_wqk[l, k * 128:(k + 1) * 128, 0:D])
                    wq_sb.append(t)
                for k in range(KD):
                    t = pwsm.tile([128, D], f32r, tag="wsm")
                    nc.sync.dma_start(t[:], d_wqk[l, k * 128:(k + 1) * 128, D:2 * D])
                    wk_sb.append(t)

                qk_sb = []
                for m in range(12):
                    w = wq_sb if m < 6 else wk_sb
                    mm = m % 6
                    ps = pps.tile([128, NT], f32, tag="mm")
                    for k in range(KD):
                        mmr(ps[:], w[k][:, mm * 128:(mm + 1) * 128], xt[k][:],
                            start=(k == 0), stop=(k == KD - 1))
                    sb = pqk.tile([128, NT], f32r, tag="qk")
                    nc.vector.tensor_scalar_add(sb[:], ps[:],
                                                pp_sb[:, 30 + m:31 + m])
                    qk_sb.append(sb)

                # ---- V projection (natural layout, per-batch token tiles) ----
                wv_sb = []
                for k in range(KD):
                    t = pwsm.tile([128, D], f32r, tag="wsm")
                    nc.sync.dma_start(t[:], d_wv[l, k * 128:(k + 1) * 128, :])
                    wv_sb.append(t)
                if bv_nz:
                    bv_sb = pbv.tile([128, D], f32, tag="bv")
                    nc.sync.dma_start(bv_sb[:], d_bvbc[l])

                v_sb = {}
                for b in range(BC):
                    for ti, (t0, tl) in enumerate(LT):
                        g0 = b * S + t0
                        vt = pv.tile([128, 12 * 65], bf16, tag="v")
                        vt3 = vt[:, :].rearrange("p (h e) -> p h e", e=65)
                        nc.vector.memset(vt3[:tl, :, 64:65], 1.0)
                        for n in range(2):
                            ps = pps.tile([128, 384], f32, tag="mm")
                            for k in range(KD):
                                mmr(ps[:tl, :], xt[k][:, g0:g0 + tl],
                                    wv_sb[k][:, n * 384:(n + 1) * 384],
                                    start=(k == 0), stop=(k == KD - 1))
                            if bv_nz:
                                nc.vector.tensor_add(
                                    ps[:tl, :], ps[:tl, :],
                                    bv_sb[:tl, n * 384:(n + 1) * 384])
                            ps3 = ps[:, :].rearrange("p (h e) -> p h e", e=64)
                            nc.vector.tensor_scalar_add(
                                vt3[:tl, n * 6:(n + 1) * 6, 0:64],
                                ps3[:tl, :, :], 0.0)
                        v_sb[(b, ti)] = vt

                # ---- attention ----
                ocat = {}
                for b in range(BC):
                    for st in range(2):
                        ocat[(b, st)] = pocat.tile([128, D], f32r, tag="ocat",
                                                   name="ocat")

                for b in range(BC):
                    for h in range(12):
                        hq = qk_sb[h // 2]
                        hk = qk_sb[6 + h // 2]
                        hb = (h % 2) * 64
                        att = {}
                        for ti, (t0, tl) in enumerate(LT):
                            # scoresT [t, all-444-cols] (2-batch-wide rhs)
                            ps = pps.tile([128, NT], f32, tag="mm")
                            mmr(ps[:tl, :],
                                hk[hb:hb + 64, b * S + t0:b * S + t0 + tl],
                                hq[hb:hb + 64, :], start=True, stop=True)
                            # exp (cols s >= t0 only), then mask-multiply
                            esc = pescr.tile([128, S], f32, tag="escr")
                            nc.scalar.activation(
                                esc[:tl, t0:S], ps[:tl, b * S + t0:b * S + S],
                                AF.Exp, scale=ATT_SCALE)
                            at = patt.tile([128, S], bf16, tag="att")
                            nc.vector.tensor_mul(at[:tl, t0:S], esc[:tl, t0:S],
                                                 mk[(b, ti)][:tl, t0:S])
                            att[ti] = at
                        for st in range(2):
                            s0, sl = LT[st]
                            po = ppa.tile([128, 65], f32, tag="aux")
                            tis = [ti for ti, (t0, tl) in enumerate(LT)
                                   if t0 < s0 + sl]
                            for i, ti in enumerate(tis):
                                t0, tl = LT[ti]
                                nc.tensor.matmul(
                                    po[:sl, :], att[ti][:tl, s0:s0 + sl],
                                    v_sb[(b, ti)][:tl, h * 65:(h + 1) * 65],
                                    start=(i == 0), stop=(i == len(tis) - 1))
                            rz = pst.tile([128, 1], f32, tag="rz")
                            nc.vector.reciprocal(rz[:sl, :], po[:sl, 64:65])
                            nc.vector.tensor_scalar_mul(
                                ocat[(b, st)][:sl, h * 64:(h + 1) * 64],
                                po[:sl, 0:64], rz[:sl, :])

                # ---- transpose o + residual add -> x2t ----
                x2t = [px.tile([128, NT], f32r, tag="x", name="x2t")
                       for _ in range(KD)]
                for b in range(BC):
                    for st in range(2):
                        s0, sl = LT[st]
                        g0 = b * S + s0
                        for k in range(KD):
                            pt = ppa.tile([128, 128], f32r, tag="aux")
                            nc.tensor.transpose(
                                pt[:, :sl],
                                ocat[(b, st)][:sl, k * 128:(k + 1) * 128],
                                ident_sb[:sl, :sl])
                            nc.vector.tensor_add(x2t[k][:, g0:g0 + sl],
                                                 xt[k][:, g0:g0 + sl],
                                                 pt[:, :sl])

                def layernorm(src, s_col, b_col):
                    """src: KD tiles [128,NT] f32 -> new normalized tiles."""
                    ps_mu = pps.tile([1, NT], f32, tag="mm")
                    for k in range(KD):
                        mmr(ps_mu[:], ones_sb[:, 0:1], src[k][:],
                            start=(k == 0), stop=(k == KD - 1))
                    ps_sq = pps.tile([1, NT], f32, tag="mm")
                    for k in range(KD):
                        s = plns.tile([128, NT], f32r, tag="lnscr")
                        nc.vector.tensor_mul(s[:], src[k][:], src[k][:])
                        mmr(ps_sq[:], ones_sb[:, 0:1], s[:],
                            start=(k == 0), stop=(k == KD - 1))
                    mu = pst.tile([1, NT], f32, tag="row")
                    nc.vector.tensor_scalar_mul(mu[:], ps_mu[:], 1.0 / D)
                    var = pst.tile([1, NT], f32, tag="row")
                    nc.vector.tensor_scalar_mul(var[:], ps_sq[:], 1.0 / D)
                    musq = pst.tile([1, NT], f32, tag="row")
                    nc.vector.tensor_mul(musq[:], mu[:], mu[:])
                    nc.vector.tensor_sub(var[:], var[:], musq[:])
                    sd = pst.tile([1, NT], f32, tag="row")
                    nc.scalar.activation(sd[:], var[:], AF.Sqrt,
                                         bias=eps_sb[:1, :])
                    rinv = pst.tile([1, NT], f32r, tag="row")
                    nc.vector.reciprocal(rinv[:], sd[:])
                    cc = pst.tile([1, NT], f32r, tag="row")
                    nc.vector.tensor_mul(cc[:], mu[:], rinv[:])
                    nc.vector.tensor_scalar_mul(cc[:], cc[:], -1.0)
                    pa = pps.tile([128, NT], f32, tag="mm")
                    mmr(pa[:], ones_sb[0:1, :], rinv[:], start=True, stop=True)
                    pc = pps.tile([128, NT], f32, tag="mm")
                    mmr(pc[:], ones_sb[0:1, :], cc[:], start=True, stop=True)
                    out = []
                    for k in range(KD):
                        t1 = plns.tile([128, NT], f32, tag="lnscr")
                        nc.vector.tensor_mul(t1[:], src[k][:], pa[:])
                        nc.vector.tensor_add(t1[:], t1[:], pc[:])
                        y = px.tile([128, NT], f32r, tag="x")
                        nc.vector.tensor_scalar(y[:], t1[:],
                                                pp_sb[:, s_col + k:s_col + k + 1],
                                                pp_sb[:, b_col + k:b_col + k + 1],
                                                OP.mult, OP.add)
                        out.append(y)
                    return out

                y1t = layernorm(x2t, 0, 6)

                # ---- FFN1 (+bias+relu, bf16 out) ----
                ht = []
                for half in range(2):
                    w1h = []
                    for k in range(KD):
                        t = pwmed.tile([128, F // 2], f32r, tag="wmed")
                        nc.sync.dma_start(
                            t[:], d_w1[l, k * 128:(k + 1) * 128,
                                       half * (F // 2):(half + 1) * (F // 2)])
                        w1h.append(t)
                    for m in range(12):
                        fm = half * 12 + m
                        ps = pps.tile([128, NT], f32, tag="mm")
                        for k in range(KD):
                            mmr(ps[:], w1h[k][:, m * 128:(m + 1) * 128],
                                y1t[k][:], start=(k == 0), stop=(k == KD - 1))
                        hb16 = ph.tile([128, NT], bf16, tag="h")
                        nc.vector.tensor_scalar(hb16[:], ps[:],
                                                pp_sb[:, 42 + fm:43 + fm], 0.0,
                                                OP.add, OP.max)
                        ht.append(hb16)

                # ---- FFN2 (bf16) + bias + residual -> x3t ----
                x3t = []
                for half in range(2):
                    w2h = []
                    for k in range(KF):
                        t = pw2.tile([128, 384], bf16, tag="w2")
                        nc.sync.dma_start(
                            t[:], d_w2[l, k * 128:(k + 1) * 128,
                                       half * 384:(half + 1) * 384])
                        w2h.append(t)
                    for m in range(3):
                        dm = half * 3 + m
                        ps = pps.tile([128, NT], f32, tag="mm")
                        for k in range(KF):
                            nc.tensor.matmul(ps[:],
                                             w2h[k][:, m * 128:(m + 1) * 128],
                                             ht[k][:], start=(k == 0),
                                             stop=(k == KF - 1))
                        x3 = px.tile([128, NT], f32r, tag="x")
                        nc.vector.tensor_add(x3[:], ps[:], y1t[dm][:])
                        if b2_nz:
                            nc.vector.tensor_scalar_add(
                                x3[:], x3[:], pp_sb[:, 24 + dm:25 + dm])
                        x3t.append(x3)

                xt = layernorm(x3t, 12, 18)

            # ship final x (bf16) to DRAM for the AllGather
            for k in range(KD):
                xb = plns.tile([128, NT], bf16, tag="xb16")
                nc.vector.tensor_copy(xb[:], xt[k][:])
                nc.sync.dma_start(ag_in[k * 128:(k + 1) * 128, :], xb[:])

        # ================= final: AllGather + vocab-sharded projection ======
        with ExitStack() as fin:
            nc.gpsimd.collective_compute(
                "AllGather", mybir.AluOpType.bypass,
                replica_groups=[list(range(NCORE))],
                ins=[ag_in.opt()], outs=[ag_out.opt()])

            pwo = fin.enter_context(tc.tile_pool(name="pwo", bufs=7))
            pxa = fin.enter_context(tc.tile_pool(name="pxa", bufs=14))
            pstrip = fin.enter_context(tc.tile_pool(name="pstrip", bufs=4))
            pstg = fin.enter_context(tc.tile_pool(name="pstg", bufs=6))
            pz = fin.enter_context(tc.tile_pool(name="pz", bufs=N_MT + 2))
            pzr = fin.enter_context(tc.tile_pool(name="pzr", bufs=6))
            pbo = (fin.enter_context(tc.tile_pool(name="pbo", bufs=1))
                   if bout_nz else None)

            wo_sb = []
            for k in range(KD):
                t = pwo.tile([128, VC], bf16, tag="wo")
                nc.sync.dma_start(t[:], d_wo[k * 128:(k + 1) * 128, :])
                wo_sb.append(t)
            if bout_nz:
                bo_sb = pbo.tile([128, VC], f32, tag="bo")
                nc.sync.dma_start(bo_sb[:], d_bout[:])

            zg_sb = pzr.tile([128, N_MT], f32, tag="zg")
            nc.vector.memset(zg_sb[:], 1.0)
            rz_sb = pzr.tile([128, N_MT], f32, tag="rzf")

            strips = {}
            zp = {}
            for mt in range(N_MT):
                c, j = mt // 4, mt % 4
                m0, ml = MT_LOC[j]
                xa = []
                for k in range(KD):
                    t = pxa.tile([128, 128], bf16, tag="xa")
                    nc.sync.dma_start(t[:, :ml],
                                      ag_out[c, k * 128:(k + 1) * 128,
                                             m0:m0 + ml])
                    xa.append(t)
                strip = pstrip.tile([128, VC], f32, tag="strip")
                strips[mt] = strip
                zpt = pz.tile([128, len(VCH)], f32, tag="zp")
                zp[mt] = zpt
                for vi, (v0, vw) in enumerate(VCH):
                    ps = ppa.tile([128, 512], f32, tag="aux")
                    for k in range(KD):
                        nc.tensor.matmul(ps[:ml, :vw], xa[k][:, :ml],
                                         wo_sb[k][:, v0:v0 + vw],
                                         start=(k == 0), stop=(k == KD - 1))
                    if bout_nz:
                        nc.vector.tensor_add(ps[:ml, :vw], ps[:ml, :vw],
                                             bo_sb[:ml, v0:v0 + vw])
                    nc.scalar.activation(strip[:ml, v0:v0 + vw], ps[:ml, :vw],
                                         AF.Exp,
                                         accum_out=zpt[:ml, vi:vi + 1])
                nc.vector.tensor_reduce(zg_sb[:ml, mt:mt + 1], zpt[:ml, :],
                                        AX.X, OP.add)

                if mt % ZGRP == ZGRP - 1:
                    g = mt // ZGRP
                    nc.sync.dma_start(z_in[g][:],
                                      zg_sb[:, g * ZGRP:(g + 1) * ZGRP])
                    nc.gpsimd.collective_compute(
                        "AllReduce", mybir.AluOpType.add,
                        replica_groups=[list(range(NCORE))],
                        ins=[z_in[g].opt()], outs=[z_out[g].opt()])
                    zr = pzr.tile([128, ZGRP], f32, tag="zred")
                    nc.sync.dma_start(zr[:], z_out[g][:])
                    nc.vector.tensor_scalar_add(zr[:], zr[:], -float(N_VPAD))
                    nc.vector.reciprocal(
                        rz_sb[:, g * ZGRP:(g + 1) * ZGRP], zr[:])
                    for mt2 in range(g * ZGRP, (g + 1) * ZGRP):
                        c2, j2 = mt2 // 4, mt2 % 4
                        m02, ml2 = MT_LOC[j2]
                        r0 = c2 * NT + m02
                        for (v0, vw) in VCH:
                            so = pstg.tile([128, 512], f32, tag="stg")
                            nc.vector.tensor_scalar_mul(
                                so[:ml2, :vw],
                                strips[mt2][:ml2, v0:v0 + vw],
                                rz_sb[:ml2, mt2:mt2 + 1])
                            nc.sync.dma_start(d_out[r0:r0 + ml2, v0:v0 + vw],
                                              so[:ml2, :vw])
                        del strips[mt2]

    nc.compile()
    return nc


def _get_nc(flags):
    if flags not in _CACHE:
        _CACHE[flags] = _build(flags)
    return _CACHE[flags]


def _prep(inputs):
    """Host-side preprocessing -> (per-core in_maps, specialization flags)."""
    x_img = np.asarray(inputs["image_token"], np.float32)
    tok = np.asarray(inputs["text_token"])
    tmask = np.asarray(inputs["text_mask"])
    temb = np.asarray(inputs["text_emb"], np.float32)
    semb = np.asarray(inputs["sep_emb"], np.float32)
    Wq = np.asarray(inputs["Wq"], np.float32)
    bq = np.asarray(inputs["bq"], np.float32)
    Wk = np.asarray(inputs["Wk"], np.float32)
    bk = np.asarray(inputs["bk"], np.float32)
    Wv = np.asarray(inputs["Wv"], np.float32)
    bv = np.asarray(inputs["bv"], np.float32)
    ln1_s = np.asarray(inputs["ln1_s"], np.float32)
    ln1_b = np.asarray(inputs["ln1_b"], np.float32)
    W1 = np.asarray(inputs["W1"], np.float32)
    b1 = np.asarray(inputs["b1"], np.float32)
    W2 = np.asarray(inputs["W2"], np.float32)
    b2 = np.asarray(inputs["b2"], np.float32)
    ln2_s = np.asarray(inputs["ln2_s"], np.float32)
    ln2_b = np.asarray(inputs["ln2_b"], np.float32)
    Wout = np.asarray(inputs["Wout"], np.float32)
    bout = np.asarray(inputs["bout"], np.float32)

    # x0 = [img | sep | emb[tokens]]
    x0 = np.concatenate(
        [x_img, np.broadcast_to(semb[None], (B, 1, D)), temb[tok]], axis=1)

    # maskT[b][t,s] = (t<=s) & combined[b,t]
    comb = np.concatenate(
        [np.ones((B, S - TXT), bool), tmask.astype(bool)], axis=1)
    tril_t = np.tril(np.ones((S, S), bool)).T  # [t,s]: t<=s
    maskt = (tril_t[None] & comb[:, :, None]).astype(ml_dtypes.bfloat16)

    # packed weights
    wqk = np.ascontiguousarray(np.concatenate([
        Wq.transpose(0, 2, 1, 3).reshape(L, D, D),
        Wk.transpose(0, 2, 1, 3).reshape(L, D, D)], axis=2))
    wv = np.ascontiguousarray(Wv.transpose(0, 2, 1, 3).reshape(L, D, D))
    w2 = W2.astype(ml_dtypes.bfloat16)

    # per-partition params: [L,128,72]
    pp = np.zeros((L, 128, 72), np.float32)

    def put(dst0, arr):  # arr [L, n*128]
        n = arr.shape[1] // 128
        pp[:, :, dst0:dst0 + n] = arr.reshape(L, n, 128).transpose(0, 2, 1)

    put(0, ln1_s); put(6, ln1_b); put(12, ln2_s); put(18, ln2_b); put(24, b2)
    put(30, np.concatenate([bq.reshape(L, D), bk.reshape(L, D)], axis=1))
    put(42, b1)

    wo_pad = np.zeros((D, V_PAD), ml_dtypes.bfloat16)
    wo_pad[:, :V] = Wout.astype(ml_dtypes.bfloat16)
    bout_pad = np.zeros(V_PAD, np.float32)
    bout_pad[:V] = bout

    flags = (bool(np.any(bv)), bool(np.any(b2)), bool(np.any(bout)))

    ones = np.ones((128, 128), np.float32)
    ident = np.eye(128, dtype=np.float32)

    in_maps = []
    for c in range(NCORE):
        m = {
            "x0t": np.ascontiguousarray(
                x0[c * BC:(c + 1) * BC].reshape(NT, D).T),
            "wqk": wqk, "wv": wv, "w1": W1, "w2": w2,
            "wout": np.ascontiguousarray(wo_pad[:, c * VC:(c + 1) * VC]),
            "pp": pp,
            "maskt": np.ascontiguousarray(maskt[c * BC:(c + 1) * BC]),
            "ones": ones, "ident": ident,
        }
        if flags[0]:
            m["bvbc"] = np.ascontiguousarray(np.broadcast_to(
                bv.reshape(L, 1, D), (L, 128, D)))
        if flags[2]:
            m["boutbc"] = np.ascontiguousarray(np.broadcast_to(
                bout_pad[c * VC:(c + 1) * VC][None], (128, VC)))
        in_maps.append(m)
    return in_maps, flags


def kernel(**inputs):
    from concourse.bass_utils import run_bass_kernel_spmd
    in_maps, flags = _prep(inputs)
    nc = _get_nc(flags)
    res = run_bass_kernel_spmd(nc, in_maps, list(range(NCORE)))
    full = np.concatenate([res.results[c]["out"] for c in range(NCORE)], axis=1)
    return np.ascontiguousarray(full[:, :V].reshape(B, S, V))
